# revision 3
# baseline (speedup 1.0000x reference)
"""TRN2 Bass kernel for CausalSCMLayer: z_causal = z @ (I - tril(A_raw,-1))^{-1}.

Math: A = tril(A_raw, -1) is strictly lower triangular (nilpotent), so
W = (I - A)^{-1} = I + R with R strictly lower triangular. out = z + z @ R.
R is tiny (256x256) and shared across the whole batch, so it is computed
exactly on the host (float64 inverse) and passed in as an input; the device
does only the batched streaming work.

The correction z @ R runs on the PE in bfloat16 (both operands rounded to
bf16, fp32 accumulate). Since it only touches the small correction term
(|z @ R| ~ 0.1 vs |z| ~ 1) the end-to-end relative error is ~3e-4; z
itself is added back in exact fp32 on the DVE. bf16 halves the PE
transpose cost (1 cycle/row vs 2 for fp32) and the PSUM->SBUF round-copy
bytes, keeping every compute engine below the DMA roofline.

Layout: z/out are moved in 1 MiB supertiles rearranged "(s p n) v" so each
SBUF partition line is one contiguous 8 KiB HBM span -> 128 descriptors
per DMA instead of 1024, which makes HWDGE descriptor generation ~8x
faster and lets the 16 DMA engines saturate from the start.

Sharding: data-parallel over the batch axis across 8 cores; R replicated.
"""

import numpy as np

import concourse.bass as bass
import concourse.tile as tile
from concourse import bacc, mybir
from concourse.bass_utils import run_bass_kernel_spmd
from concourse.masks import make_identity

F32 = mybir.dt.float32
BF16 = mybir.dt.bfloat16

N_CORES = 8
BATCH = 131072
NVARS = 256
BC = BATCH // N_CORES          # rows per core
TILES_PER_DMA = 8              # 8 x 128 rows = 1MiB per DMA
ROWS_PER_DMA = TILES_PER_DMA * 128
N_SUPER = BC // ROWS_PER_DMA   # outer loop count

_CACHE = {}


def _build_nc():
    nc = bacc.Bacc("TRN2", target_bir_lowering=False, debug=False,
                   num_devices=N_CORES)
    z = nc.dram_tensor("z", [BC, NVARS], F32, kind="ExternalInput").ap()
    r = nc.dram_tensor("r", [128, 2 * NVARS], F32, kind="ExternalInput").ap()
    out = nc.dram_tensor("out", [BC, NVARS], F32, kind="ExternalOutput").ap()

    # (s p n): within a 1024-row supertile, partition p holds rows
    # p*8 .. p*8+7 — one contiguous 8 KiB HBM chunk per partition.
    # Row<->partition mapping is identical on load and store, and every
    # batch row is independent, so compute tiles n are just a different
    # (consistent) 128-row subset than with the (s n p) layout.
    z_r = z.rearrange("(s p n) v -> s p n v", p=128, n=TILES_PER_DMA)
    o_r = out.rearrange("(s p n) v -> s p n v", p=128, n=TILES_PER_DMA)

    with tile.TileContext(nc) as tc:
        with (
            tc.tile_pool(name="const", bufs=1) as cp,
            tc.tile_pool(name="zin", bufs=12) as zin_pool,
            tc.tile_pool(name="outb", bufs=8) as outb_pool,
            tc.tile_pool(name="zb", bufs=8) as zb_pool,
            tc.tile_pool(name="ztr", bufs=16) as ztr_pool,
            tc.tile_pool(name="psT", bufs=2, space="PSUM") as psT_pool,
            tc.tile_pool(name="psC", bufs=4, space="PSUM") as psC_pool,
            tc.tile_pool(name="psW", bufs=1, space="PSUM") as psW_pool,
        ):
            ident = cp.tile([128, 128], F32)
            identb = cp.tile([128, 128], BF16)
            Rm = cp.tile([128, 2 * NVARS], F32)
            Rmb = cp.tile([128, 2 * NVARS], BF16)
            # R first in the SP HWDGE ring: lands ~2us, well before the
            # first matmul needs it.
            nc.sync.dma_start(Rm[:], r)
            make_identity(nc, ident[:])
            nc.vector.tensor_copy(identb[:], ident[:])
            nc.vector.tensor_copy(Rmb[:], Rm[:])
            Rmb0 = Rmb[:, 0:256]    # rows 0:128 of R, bf16
            Rmb1 = Rmb[:, 256:512]  # rows 128:256 of R, bf16

            # PE p-state warm-up: HAM starts the PE clock-gated at 1.2 GHz
            # and only un-throttles after ~3.4us of sustained activity.
            # Burn the load-wait window with dep-free transposes so real
            # work runs at 2.4 GHz.
            warm = psW_pool.tile([128, 128], BF16, tag="warm", name="warmps")
            for _ in range(16):
                nc.tensor.transpose(warm[:], identb[:], identb[:])

            # main loop: out = z + z @ R, 128-row tiles, software-pipelined
            # by SKEW tiles so the PE never stalls on the ACT round-copy.
            zin_t = {}
            outb_t = {}
            work = []
            for s in range(N_SUPER):
                zin_t[s] = zin_pool.tile([128, TILES_PER_DMA, 256], F32,
                                         tag="zin", name=f"zin{s}")
                nc.sync.dma_start(zin_t[s][:], z_r[s])
                outb_t[s] = outb_pool.tile([128, TILES_PER_DMA, 256], F32,
                                           tag="outb", name=f"outb{s}")
                for n in range(TILES_PER_DMA):
                    work.append((s, n))

            from collections import deque
            SKEW = 3  # transposes run 3 tiles ahead of the matmuls
            pending = deque()
            done_in_super = {s: 0 for s in range(N_SUPER)}

            def flush(p):
                zr, zt, out_ap, s = p
                pC = psC_pool.tile([128, 256], F32, tag="pC", name=f"pC{s}")
                nc.tensor.matmul(pC[:], zr[:, 0:128], Rmb0,
                                 start=True, stop=False)
                nc.tensor.matmul(pC[:], zr[:, 128:256], Rmb1,
                                 start=False, stop=True)
                nc.vector.tensor_add(out_ap, zt, pC[:])
                done_in_super[s] += 1
                h = TILES_PER_DMA // 2
                # first and last supertiles store in halves: the first
                # launches the store stream ~2us earlier, the last
                # overlaps its store with the final adds.
                split = s < 2 or s == N_SUPER - 1
                if split and done_in_super[s] == h:
                    nc.gpsimd.dma_start(o_r[s][:, 0:h, :], outb_t[s][:, 0:h, :])
                elif split and done_in_super[s] == TILES_PER_DMA:
                    nc.gpsimd.dma_start(o_r[s][:, h:, :], outb_t[s][:, h:, :])
                elif done_in_super[s] == TILES_PER_DMA:
                    nc.gpsimd.dma_start(o_r[s], outb_t[s][:])

            for ti, (s, n) in enumerate(work):
                zt = zin_t[s][:, n, :]
                # Pool pre-rounds z to bf16 so the PE transpose runs at
                # 1 cycle/row instead of 2 (fp32); the exact fp32 z is
                # still what the DVE adds back at the end.
                zb = zb_pool.tile([128, 256], BF16, tag="zb", name=f"zb{s}_{n}")
                nc.gpsimd.tensor_copy(zb[:], zt)
                pT = psT_pool.tile([128, 256], BF16, tag="pT", name=f"pT{s}_{n}")
                nc.tensor.transpose(pT[:, 0:128], zb[:, 0:128], identb[:])
                nc.tensor.transpose(pT[:, 128:256], zb[:, 128:256], identb[:])
                zr = ztr_pool.tile([128, 256], BF16, tag="zr", name=f"zr{s}_{n}")
                nc.scalar.copy(zr[:], pT[:])
                pending.append((zr, zt, outb_t[s][:, n, :], s))
                if len(pending) > SKEW:
                    flush(pending.popleft())
            while pending:
                flush(pending.popleft())

    nc.compile()
    return nc


def _get_nc():
    if "nc" not in _CACHE:
        _CACHE["nc"] = _build_nc()
    return _CACHE["nc"]


def kernel(z_exogenous, A_raw):
    # NTFF tracing needs antenv.axon_hooks; if BASS_TRACE is set in an
    # environment that lacks it, run_bass_kernel_spmd would crash.
    import os
    try:
        import antenv.axon_hooks  # noqa: F401
    except ImportError:
        os.environ["BASS_NEVER_TRACE"] = "1"

    z = np.ascontiguousarray(np.asarray(z_exogenous, dtype=np.float32))
    A = np.ascontiguousarray(np.asarray(A_raw, dtype=np.float32))
    assert z.shape == (BATCH, NVARS) and A.shape == (NVARS, NVARS)

    # R = (I - A)^{-1} - I, computed exactly in float64 on the host
    # (256x256, ~microseconds) and packed as [rows 0:128 | rows 128:256].
    A64 = np.tril(A.astype(np.float64), -1)
    eye = np.eye(NVARS, dtype=np.float64)
    R = (np.linalg.inv(eye - A64) - eye).astype(np.float32)
    Rm = np.ascontiguousarray(
        np.concatenate([R[0:128, :], R[128:256, :]], axis=1))

    nc = _get_nc()
    in_maps = [
        {"z": z[i * BC:(i + 1) * BC], "r": Rm} for i in range(N_CORES)
    ]
    res = run_bass_kernel_spmd(nc, in_maps, core_ids=list(range(N_CORES)))
    kernel.last_exec_time_ns = res.exec_time_ns
    kernel.last_results = res
    return np.concatenate([res.results[i]["out"] for i in range(N_CORES)], axis=0)


# revision 4
# speedup vs baseline: 1.4977x; 1.4977x over previous
"""TRN2 Bass kernel for CausalSCMLayer: z_causal = z @ (I - tril(A_raw,-1))^{-1}.

Math: A = tril(A_raw, -1) is strictly lower triangular (nilpotent), so
W = (I - A)^{-1} is unit lower triangular, tiny (256x256), and shared
across the whole batch. It is computed exactly on the host (float64
inverse) and passed in as an input; the device does only the batched
streaming work out = z @ W.

The batched matmul runs on the PE in float32r (TF32-like, ~11-bit
mantissa, exact products, fp32 accumulate): each 128-row tile of z is
transposed on the PE (fp32, exact), rounded to f32r by the ACT engine's
PSUM->SBUF round-copy, multiplied by the f32r W, and the fp32 PSUM result
is copied out by the DVE. End-to-end error ~1e-4, dominated by the tf32
rounding of z itself.

Layout: z/out are moved in 1 MiB supertiles rearranged "(s p n) v" so each
SBUF partition line is one contiguous 8 KiB HBM span -> 128 descriptors
per DMA instead of 1024, which makes HWDGE descriptor generation ~8x
faster and lets the 16 DMA engines saturate from the start.

Sharding: data-parallel over the batch axis across 8 cores; W replicated.
"""

import numpy as np

import concourse.bass as bass
import concourse.tile as tile
from concourse import bacc, mybir
from concourse.bass_utils import run_bass_kernel_spmd
from concourse.masks import make_identity

F32 = mybir.dt.float32
F32R = mybir.dt.float32r

N_CORES = 8
BATCH = 131072
NVARS = 256
BC = BATCH // N_CORES          # rows per core
TILES_PER_DMA = 8              # 8 x 128 rows = 1MiB per DMA
ROWS_PER_DMA = TILES_PER_DMA * 128
N_SUPER = BC // ROWS_PER_DMA   # outer loop count

_CACHE = {}


def _build_nc():
    nc = bacc.Bacc("TRN2", target_bir_lowering=False, debug=False,
                   num_devices=N_CORES)
    z = nc.dram_tensor("z", [BC, NVARS], F32, kind="ExternalInput").ap()
    r = nc.dram_tensor("r", [128, 2 * NVARS], F32, kind="ExternalInput").ap()
    out = nc.dram_tensor("out", [BC, NVARS], F32, kind="ExternalOutput").ap()

    # (s p n): within a 1024-row supertile, partition p holds rows
    # p*8 .. p*8+7 — one contiguous 8 KiB HBM chunk per partition.
    # Row<->partition mapping is identical on load and store, and every
    # batch row is independent, so compute tiles n are just a different
    # (consistent) 128-row subset than with the (s n p) layout.
    z_r = z.rearrange("(s p n) v -> s p n v", p=128, n=TILES_PER_DMA)
    o_r = out.rearrange("(s p n) v -> s p n v", p=128, n=TILES_PER_DMA)

    with tile.TileContext(nc) as tc:
        with (
            tc.tile_pool(name="const", bufs=1) as cp,
            tc.tile_pool(name="zin", bufs=12) as zin_pool,
            tc.tile_pool(name="outb", bufs=8) as outb_pool,
            tc.tile_pool(name="ztr", bufs=16) as ztr_pool,
            tc.tile_pool(name="psT", bufs=2, space="PSUM") as psT_pool,
            tc.tile_pool(name="psC", bufs=4, space="PSUM") as psC_pool,
            tc.tile_pool(name="psW", bufs=1, space="PSUM") as psW_pool,
        ):
            ident = cp.tile([128, 128], F32)
            Wm = cp.tile([128, 2 * NVARS], F32)
            Wmr = cp.tile([128, 2 * NVARS], F32R)
            # W first in the SP HWDGE ring: lands ~2us, well before the
            # first matmul needs it.
            nc.sync.dma_start(Wm[:], r)
            make_identity(nc, ident[:])
            # DVE round-copy to f32r (the PE's fp32r path requires
            # pre-rounded operands).
            nc.vector.tensor_copy(Wmr[:], Wm[:])
            Wmr0 = Wmr[:, 0:256]    # rows 0:128 of W
            Wmr1 = Wmr[:, 256:512]  # rows 128:256 of W

            # PE p-state warm-up: HAM starts the PE clock-gated at 1.2 GHz
            # and only un-throttles after ~3.4us of sustained activity.
            # Burn the load-wait window with dep-free transposes so real
            # work runs at 2.4 GHz.
            warm = psW_pool.tile([128, 128], F32, tag="warm", name="warmps")
            for _ in range(10):
                nc.tensor.transpose(warm[:], ident[:], ident[:])

            # main loop: out = z @ W, 128-row tiles, software-pipelined
            # by SKEW tiles so the PE never stalls on the ACT round-copy.
            zin_t = {}
            outb_t = {}
            work = []
            for s in range(N_SUPER):
                zin_t[s] = zin_pool.tile([128, TILES_PER_DMA, 256], F32,
                                         tag="zin", name=f"zin{s}")
                nc.sync.dma_start(zin_t[s][:], z_r[s])
                outb_t[s] = outb_pool.tile([128, TILES_PER_DMA, 256], F32,
                                           tag="outb", name=f"outb{s}")
                for n in range(TILES_PER_DMA):
                    work.append((s, n))

            from collections import deque
            SKEW = 3  # transposes run 3 tiles ahead of the matmuls
            pending = deque()
            done_in_super = {s: 0 for s in range(N_SUPER)}

            def flush(p):
                zr, out_ap, s = p
                pC = psC_pool.tile([128, 256], F32, tag="pC", name=f"pC{s}")
                nc.tensor.matmul(pC[:], zr[:, 0:128], Wmr0,
                                 start=True, stop=False)
                nc.tensor.matmul(pC[:], zr[:, 128:256], Wmr1,
                                 start=False, stop=True)
                nc.vector.tensor_copy(out_ap, pC[:])
                done_in_super[s] += 1
                h = TILES_PER_DMA // 2
                # first and last supertiles store in halves: the first
                # launches the store stream ~2us earlier, the last
                # overlaps its store with the final copies.
                split = s < 2 or s == N_SUPER - 1
                if split and done_in_super[s] == h:
                    nc.gpsimd.dma_start(o_r[s][:, 0:h, :], outb_t[s][:, 0:h, :])
                elif split and done_in_super[s] == TILES_PER_DMA:
                    nc.gpsimd.dma_start(o_r[s][:, h:, :], outb_t[s][:, h:, :])
                elif done_in_super[s] == TILES_PER_DMA:
                    nc.gpsimd.dma_start(o_r[s], outb_t[s][:])

            for ti, (s, n) in enumerate(work):
                zt = zin_t[s][:, n, :]
                pT = psT_pool.tile([128, 256], F32, tag="pT", name=f"pT{s}_{n}")
                nc.tensor.transpose(pT[:, 0:128], zt[:, 0:128], ident[:])
                nc.tensor.transpose(pT[:, 128:256], zt[:, 128:256], ident[:])
                zr = ztr_pool.tile([128, 256], F32R, tag="zr", name=f"zr{s}_{n}")
                nc.scalar.copy(zr[:], pT[:])
                pending.append((zr, outb_t[s][:, n, :], s))
                if len(pending) > SKEW:
                    flush(pending.popleft())
            while pending:
                flush(pending.popleft())

    nc.compile()
    return nc


def _get_nc():
    if "nc" not in _CACHE:
        _CACHE["nc"] = _build_nc()
    return _CACHE["nc"]


def kernel(z_exogenous, A_raw):
    # NTFF tracing needs antenv.axon_hooks; if BASS_TRACE is set in an
    # environment that lacks it, run_bass_kernel_spmd would crash.
    import os
    try:
        import antenv.axon_hooks  # noqa: F401
    except ImportError:
        os.environ["BASS_NEVER_TRACE"] = "1"

    z = np.ascontiguousarray(np.asarray(z_exogenous, dtype=np.float32))
    A = np.ascontiguousarray(np.asarray(A_raw, dtype=np.float32))
    assert z.shape == (BATCH, NVARS) and A.shape == (NVARS, NVARS)

    # W = (I - A)^{-1}, computed exactly in float64 on the host
    # (256x256, ~microseconds) and packed as [rows 0:128 | rows 128:256].
    A64 = np.tril(A.astype(np.float64), -1)
    eye = np.eye(NVARS, dtype=np.float64)
    W = np.linalg.inv(eye - A64).astype(np.float32)
    Wm = np.ascontiguousarray(
        np.concatenate([W[0:128, :], W[128:256, :]], axis=1))

    nc = _get_nc()
    in_maps = [
        {"z": z[i * BC:(i + 1) * BC], "r": Wm} for i in range(N_CORES)
    ]
    res = run_bass_kernel_spmd(nc, in_maps, core_ids=list(range(N_CORES)))
    kernel.last_exec_time_ns = res.exec_time_ns
    kernel.last_results = res
    return np.concatenate([res.results[i]["out"] for i in range(N_CORES)], axis=0)


# revision 7
# speedup vs baseline: 1.5963x; 1.0658x over previous
"""TRN2 Bass kernel for CausalSCMLayer: z_causal = z @ (I - tril(A_raw,-1))^{-1}.

Math: A = tril(A_raw, -1) is strictly lower triangular (nilpotent), so
W = (I - A)^{-1} is unit lower triangular, tiny (256x256), and shared
across the whole batch. It is computed exactly on the host (float64
inverse) and passed in as an input; the device does only the batched
streaming work out = z @ W.

The batched matmul runs on the PE in float32r (TF32-like, ~11-bit
mantissa, exact products, fp32 accumulate): each 128-row tile of z is
transposed on the PE (fp32, exact), rounded to f32r by the ACT engine's
PSUM->SBUF round-copy, multiplied by the f32r W, and the fp32 PSUM result
is copied out by the DVE. End-to-end error ~1e-4, dominated by the tf32
rounding of z itself.

DMA: z/out move in variable-size supertiles rearranged "(p n) v" so each
SBUF partition line is one contiguous >=8 KiB HBM span (128 descriptors
per DMA). Per-queue throughput is flat at ~26.5 B/ns for descriptors
>=4 KiB, but every DMA instruction serializes a ~350 ns completion
descriptor on queue 0 (loads) / queue 15 (stores), so the instruction
count is kept minimal: two 1 MiB starter loads for a fast pipeline ramp,
then 2 MiB supertiles; stores split only for the first supertile (to
launch the store stream early) and the last (to shorten the tail).

Sharding: data-parallel over the batch axis across 8 cores; W replicated.
"""

import numpy as np

import concourse.bass as bass
import concourse.tile as tile
from concourse import bacc, mybir
from concourse.bass_utils import run_bass_kernel_spmd
from concourse.masks import make_identity

F32 = mybir.dt.float32
F32R = mybir.dt.float32r

N_CORES = 8
BATCH = 131072
NVARS = 256
BC = BATCH // N_CORES          # rows per core
# supertile sizes in 128-row tiles (sum = BC/128 = 128 tiles)
SUPER_TILES = [8, 8, 16, 16, 16, 16, 16, 16, 16]
assert sum(SUPER_TILES) * 128 == BC

_CACHE = {}


def _build_nc():
    nc = bacc.Bacc("TRN2", target_bir_lowering=False, debug=False,
                   num_devices=N_CORES)
    z = nc.dram_tensor("z", [BC, NVARS], F32, kind="ExternalInput").ap()
    r = nc.dram_tensor("r", [128, 2 * NVARS], F32, kind="ExternalInput").ap()
    out = nc.dram_tensor("out", [BC, NVARS], F32, kind="ExternalOutput").ap()

    # (p n) within a supertile: partition p holds rows p*T .. p*T+T-1 —
    # one contiguous T KiB HBM chunk per partition. Row<->partition
    # mapping is identical on load and store and every batch row is
    # independent, so compute tiles n are just a consistent 128-row
    # subset.
    def z_sup(row0, T):
        return z[row0:row0 + T * 128, :].rearrange(
            "(p n) v -> p n v", p=128, n=T)

    def o_sup(row0, T):
        return out[row0:row0 + T * 128, :].rearrange(
            "(p n) v -> p n v", p=128, n=T)

    with tile.TileContext(nc) as tc:
        with (
            tc.tile_pool(name="const", bufs=1) as cp,
            tc.tile_pool(name="zin8", bufs=2) as zin8_pool,
            tc.tile_pool(name="zin16", bufs=4) as zin16_pool,
            tc.tile_pool(name="outb8", bufs=2) as outb8_pool,
            tc.tile_pool(name="outb16", bufs=4) as outb16_pool,
            tc.tile_pool(name="ztr", bufs=16) as ztr_pool,
            tc.tile_pool(name="psT", bufs=3, space="PSUM") as psT_pool,
            tc.tile_pool(name="psC", bufs=5, space="PSUM") as psC_pool,
        ):
            ident = cp.tile([128, 128], F32)
            Wm = cp.tile([128, 2 * NVARS], F32)
            Wmr = cp.tile([128, 2 * NVARS], F32R)
            # W first in the SP HWDGE ring: lands ~2us, well before the
            # first matmul needs it.
            nc.sync.dma_start(Wm[:], r)
            make_identity(nc, ident[:])
            # DVE round-copy to f32r (the PE's fp32r path requires
            # pre-rounded operands).
            nc.vector.tensor_copy(Wmr[:], Wm[:])
            Wmr0 = Wmr[:, 0:256]    # rows 0:128 of W
            Wmr1 = Wmr[:, 256:512]  # rows 128:256 of W

            # PE p-state warm-up: HAM starts the PE clock-gated at 1.2 GHz
            # and only un-throttles after ~3.4us of sustained activity.
            # Burn the load-wait window with dep-free transposes so real
            # work runs at 2.4 GHz.
            warm = psT_pool.tile([128, 256], F32, tag="pT", name="warmps")
            for _ in range(10):
                nc.tensor.transpose(warm[:, 0:128], ident[:], ident[:])

            # main loop: out = z @ W, 128-row tiles, software-pipelined
            # by SKEW tiles so the PE never stalls on the ACT round-copy.
            n_super = len(SUPER_TILES)
            zin_t = {}
            outb_t = {}
            work = []
            row0 = 0
            row0s = []
            for s, T in enumerate(SUPER_TILES):
                row0s.append(row0)
                zp = zin8_pool if T == 8 else zin16_pool
                op = outb8_pool if T == 8 else outb16_pool
                zin_t[s] = zp.tile([128, T, 256], F32,
                                   tag=f"zin{T}", name=f"zin{s}")
                nc.sync.dma_start(zin_t[s][:], z_sup(row0, T))
                outb_t[s] = op.tile([128, T, 256], F32,
                                    tag=f"outb{T}", name=f"outb{s}")
                for n in range(T):
                    work.append((s, n))
                row0 += T * 128

            from collections import deque
            SKEW = 4  # transposes run 4 tiles ahead of the matmuls
            pending = deque()
            done_in_super = {s: 0 for s in range(n_super)}

            def flush(p):
                zr, out_ap, s = p
                pC = psC_pool.tile([128, 256], F32, tag="pC", name=f"pC{s}")
                nc.tensor.matmul(pC[:], zr[:, 0:128], Wmr0,
                                 start=True, stop=False)
                nc.tensor.matmul(pC[:], zr[:, 128:256], Wmr1,
                                 start=False, stop=True)
                nc.vector.tensor_copy(out_ap, pC[:])
                done_in_super[s] += 1
                T = SUPER_TILES[s]
                h = T // 2
                osup = o_sup(row0s[s], T)
                # first and last supertiles store in halves: the first
                # launches the store stream ~2us earlier, the last
                # overlaps its store with the final copies.
                split = s == 0 or s == n_super - 1
                if split and done_in_super[s] == h:
                    nc.gpsimd.dma_start(osup[:, 0:h, :], outb_t[s][:, 0:h, :])
                elif split and done_in_super[s] == T:
                    nc.gpsimd.dma_start(osup[:, h:, :], outb_t[s][:, h:, :])
                elif done_in_super[s] == T:
                    nc.gpsimd.dma_start(osup, outb_t[s][:])

            for ti, (s, n) in enumerate(work):
                zt = zin_t[s][:, n, :]
                pT = psT_pool.tile([128, 256], F32, tag="pT", name=f"pT{s}_{n}")
                nc.tensor.transpose(pT[:, 0:128], zt[:, 0:128], ident[:])
                nc.tensor.transpose(pT[:, 128:256], zt[:, 128:256], ident[:])
                zr = ztr_pool.tile([128, 256], F32R, tag="zr", name=f"zr{s}_{n}")
                nc.scalar.copy(zr[:], pT[:])
                pending.append((zr, outb_t[s][:, n, :], s))
                if len(pending) > SKEW:
                    flush(pending.popleft())
            while pending:
                flush(pending.popleft())

    nc.compile()
    return nc


def _get_nc():
    if "nc" not in _CACHE:
        _CACHE["nc"] = _build_nc()
    return _CACHE["nc"]


def kernel(z_exogenous, A_raw):
    # NTFF tracing needs antenv.axon_hooks; if BASS_TRACE is set in an
    # environment that lacks it, run_bass_kernel_spmd would crash.
    import os
    try:
        import antenv.axon_hooks  # noqa: F401
    except ImportError:
        os.environ["BASS_NEVER_TRACE"] = "1"

    z = np.ascontiguousarray(np.asarray(z_exogenous, dtype=np.float32))
    A = np.ascontiguousarray(np.asarray(A_raw, dtype=np.float32))
    assert z.shape == (BATCH, NVARS) and A.shape == (NVARS, NVARS)

    # W = (I - A)^{-1}, computed exactly in float64 on the host
    # (256x256, ~microseconds) and packed as [rows 0:128 | rows 128:256].
    A64 = np.tril(A.astype(np.float64), -1)
    eye = np.eye(NVARS, dtype=np.float64)
    W = np.linalg.inv(eye - A64).astype(np.float32)
    Wm = np.ascontiguousarray(
        np.concatenate([W[0:128, :], W[128:256, :]], axis=1))

    nc = _get_nc()
    in_maps = [
        {"z": z[i * BC:(i + 1) * BC], "r": Wm} for i in range(N_CORES)
    ]
    res = run_bass_kernel_spmd(nc, in_maps, core_ids=list(range(N_CORES)))
    kernel.last_exec_time_ns = res.exec_time_ns
    kernel.last_results = res
    return np.concatenate([res.results[i]["out"] for i in range(N_CORES)], axis=0)
